# revision 1
# baseline (speedup 1.0000x reference)
"""Trainium2 Bass kernel for KGETCDA GNN message-passing layer.

Computes, for fixed-structure inputs:
    side    = segment_sum(a_vals[:,None] * ego[a_cols], a_rows, N)
    sum_emb = LeakyReLU((ego + side) @ W1.T + b1)
    bi_emb  = LeakyReLU((ego * side) @ W2.T + b2)
    out     = sum_emb + bi_emb

Strategy (8 NeuronCores, SPMD, full inputs in / full output out):
  - Shard destination rows (a_rows) contiguously: core c owns rows
    [c*N/8, (c+1)*N/8).  Edges partitioned by destination.
  - Per core, edges are sorted by destination and grouped into 512-dest
    "windows"; each 128-edge tile is turned into messages via
    gpsimd.dma_gather (per-edge descriptor DMA from a fp16 [N,128] padded
    copy of ego), and scatter-reduced into a PSUM window [96, 512] with a
    single matmul against a per-tile valued one-hot built on DVE
    (iota==dloc)*val, all in fp16 with f32 PSUM accumulation.
  - dma_gather indices are int16, so edges are split into stream A
    (src < 32768) and stream B (src >= 32768, rebased) per window.
  - Per-(window, stream) tile counts are padded to the max over the 8
    cores so the single SPMD instruction stream is valid for every core.
  - Dense tail is computed feature-major: sumXt = egoT+sideT,
    biXt = egoT*sideT (DVE, reading PSUM windows directly), then per-128-node
    chunk matmuls against bias-augmented W1T/W2T (f32), LeakyReLU on ScalarE,
    branch add on DVE, one big DMA out.
"""

import numpy as np
import ml_dtypes

import concourse.bacc as bacc
import concourse.bass as bass
import concourse.mybir as mybir
import concourse.tile as tile
from concourse import bass_utils, library_config

# ---------------------------------------------------------------- constants
N_NODES = 50000
N_EDGES = 800000
D = 96
DPAD = 128          # fp16 gather element (256B, dma_gather alignment)
NCORES = 8
PER = N_NODES // NCORES          # 6250 dests per core
WIN = 512                        # dests per PSUM window
NWIN = (PER + WIN - 1) // WIN    # 13 windows (last short)
SPLIT = 32768                    # int16 index limit for dma_gather
GT = 128                         # edges per tile
CT = 8                           # tiles per dma_gather call (ring limit ~1024 idxs)
NQ = 4                           # SWDGE queues
NCHUNK = (PER + 127) // 128      # 49 dense chunks of 128 nodes
PERPAD = NCHUNK * 128            # 6272
NEG_SLOPE = 0.01

FP16 = mybir.dt.float16
F32 = mybir.dt.float32
I16 = mybir.dt.int16


# ---------------------------------------------------------------- host prep
def _preprocess(a_rows, a_cols, a_vals):
    """Per-core edge layout with core-uniform tile counts.

    Returns (tile_plan, per_core) where tile_plan is a list of
    (window, n_valid_dest) plus per-(window,stream) tile counts TA/TB, and
    per_core[c] = dict(idx16, vals, dlocs) already tiled/padded.
    """
    a_rows = np.asarray(a_rows).astype(np.int64)
    a_cols = np.asarray(a_cols).astype(np.int64)
    a_vals = np.asarray(a_vals).astype(np.float32)

    core = a_rows // PER
    dloc_all = a_rows % PER

    # per (core, window, stream) edge lists
    counts = np.zeros((NCORES, NWIN, 2), dtype=np.int64)
    buckets = [[[None, None] for _ in range(NWIN)] for _ in range(NCORES)]
    order = np.argsort(a_rows, kind="stable")
    r_s, c_s, v_s, dl_s = a_rows[order], a_cols[order], a_vals[order], dloc_all[order]
    cr_s = core[order]
    for c in range(NCORES):
        m = cr_s == c
        dl = dl_s[m]
        src = c_s[m]
        val = v_s[m]
        w_of = dl // WIN
        stream = (src >= SPLIT).astype(np.int64)
        for w in range(NWIN):
            for s in range(2):
                mm = (w_of == w) & (stream == s)
                idx = src[mm] - (SPLIT if s else 0)
                buckets[c][w][s] = (idx, val[mm], dl[mm] % WIN)
                counts[c, w, s] = mm.sum()

    # uniform tile counts: max over cores per (window, stream)
    T = np.zeros((NWIN, 2), dtype=np.int64)
    for w in range(NWIN):
        for s in range(2):
            T[w, s] = int(np.ceil(counts[:, w, s].max() / GT))

    per_core = []
    for c in range(NCORES):
        idx_parts, val_parts, dl_parts = [], [], []
        for w in range(NWIN):
            for s in range(2):
                idx, val, dl = buckets[c][w][s]
                n_pad = int(T[w, s]) * GT
                pad = n_pad - len(idx)
                idx_parts.append(np.concatenate([idx, np.zeros(pad, np.int64)]))
                val_parts.append(np.concatenate([val, np.zeros(pad, np.float32)]))
                dl_parts.append(np.concatenate([dl, np.zeros(pad, np.int64)]))
        idx_all = np.concatenate(idx_parts)       # [TT*128]
        val_all = np.concatenate(val_parts).astype(np.float32)
        dl_all = np.concatenate(dl_parts)
        per_core.append(dict(idx=idx_all, val=val_all, dloc=dl_all))
    return T, per_core


def _build_call_plan(T):
    """Split the uniform tile sequence into dma_gather calls (<=CT tiles,
    single stream each).  Returns list of (stream, tile_start, n_tiles) in
    global tile order, plus per-tile (window, stream) labels."""
    calls = []
    tiles = []  # (window, stream) per global tile
    t = 0
    for w in range(NWIN):
        for s in range(2):
            n = int(T[w, s])
            done = 0
            while done < n:
                k = min(CT, n - done)
                calls.append((s, t + done, k))
                done += k
            for _ in range(n):
                tiles.append((w, s))
            t += n
    return calls, tiles


def _wrap_idx16(idx_all, calls):
    """Per-call 16-partition-wrapped int16 index tiles, concatenated.
    Call k with n_tiles tiles occupies columns [8*tile_start, 8*(start+n))
    of a [128, 8*TT] int16 array (8 cols per tile: 128/16)."""
    TT = len(idx_all) // GT
    out = np.zeros((128, 8 * TT), dtype=np.int16)
    for s, t0, nt in calls:
        chunk = idx_all[t0 * GT:(t0 + nt) * GT].astype(np.int16)
        wrapped = chunk.reshape(-1, 16).T          # [16, nt*8]
        out[:, t0 * 8:(t0 + nt) * 8] = np.tile(wrapped, (8, 1))
    return out


# ---------------------------------------------------------------- builder
_CACHE = {}
_LAST_RESULT = None


def _build_program(T, calls, tiles):
    TT = len(tiles)
    nc = bacc.Bacc("TRN2", target_bir_lowering=False, debug=False,
                   num_devices=NCORES, num_swdge_queues=NQ)

    ego_pad = nc.dram_tensor("ego_pad", [N_NODES, DPAD], FP16, kind="ExternalInput")
    idx16 = nc.dram_tensor("idx16", [128, 8 * TT], I16, kind="ExternalInput")
    vals = nc.dram_tensor("vals", [128, TT], F32, kind="ExternalInput")
    dlocs = nc.dram_tensor("dlocs", [128, TT], F32, kind="ExternalInput")
    iota = nc.dram_tensor("iota", [128, WIN], FP16, kind="ExternalInput")
    egot = nc.dram_tensor("egot", [D + 1, PERPAD], F32, kind="ExternalInput")
    w1t = nc.dram_tensor("w1t", [D + 1, D], F32, kind="ExternalInput")
    w2t = nc.dram_tensor("w2t", [D + 1, D], F32, kind="ExternalInput")
    out = nc.dram_tensor("out", [PERPAD, D], F32, kind="ExternalOutput")

    win_ndest = [min(WIN, PER - w * WIN) for w in range(NWIN)]
    tile2call = {}
    for ci, (s, t0, nt) in enumerate(calls):
        for j in range(nt):
            tile2call[t0 + j] = (ci, j)

    with tile.TileContext(nc) as tc:
        with tc.tile_pool(name="const", bufs=1) as constp, \
             tc.tile_pool(name="gath", bufs=6) as gathp, \
             tc.tile_pool(name="oh", bufs=4) as ohp, \
             tc.tile_pool(name="pw", bufs=3, space="PSUM") as pwp, \
             tc.tile_pool(name="pd", bufs=4, space="PSUM") as pdp, \
             tc.tile_pool(name="act", bufs=4) as actp, \
             tc.tile_pool(name="big", bufs=1) as bigp:

            # ---- constants / streams resident in SBUF
            idx_sb = constp.tile([128, 8 * TT], I16)
            nc.sync.dma_start(idx_sb[:], idx16[:])
            val_sb = constp.tile([128, TT], F32)
            nc.sync.dma_start(val_sb[:], vals[:])
            dloc_sb = constp.tile([128, TT], F32)
            nc.sync.dma_start(dloc_sb[:], dlocs[:])
            iota_sb = constp.tile([128, WIN], FP16)
            nc.sync.dma_start(iota_sb[:], iota[:])
            egot_sb = bigp.tile([D + 1, PERPAD], F32)
            nc.sync.dma_start(egot_sb[:], egot[:])
            w1t_sb = constp.tile([D + 1, D], F32)
            nc.sync.dma_start(w1t_sb[:], w1t[:])
            w2t_sb = constp.tile([D + 1, D], F32)
            nc.sync.dma_start(w2t_sb[:], w2t[:])

            sumxt = bigp.tile([D + 1, PERPAD], F32)
            bixt = bigp.tile([D + 1, PERPAD], F32)
            out_sb = bigp.tile([128, NCHUNK, D], F32)

            # ones rows for the bias augmentation
            nc.vector.memset(sumxt[D:D + 1, :], 1.0)
            nc.vector.memset(bixt[D:D + 1, :], 1.0)

            nc.gpsimd.load_library(library_config.mlp)

            # ---- gather calls (issued in order; Tile double-buffers)
            gath_tiles = [None] * len(calls)
            for ci, (s, t0, nt) in enumerate(calls):
                g = gathp.tile([128, CT, DPAD], FP16, tag="gath")
                src_ap = ego_pad[:SPLIT, :] if s == 0 else ego_pad[SPLIT:, :]
                nc.gpsimd.dma_gather(
                    g[:, :nt, :], src_ap, idx_sb[:, t0 * 8:(t0 + nt) * 8],
                    nt * GT, nt * GT, DPAD, queue_num=ci % NQ,
                )
                gath_tiles[ci] = g

            # ---- per-window accumulation + fused dense prologue
            t = 0
            for w in range(NWIN):
                nd = win_ndest[w]
                pw = pwp.tile([D, WIN], F32, tag="pw")
                n_t = int(T[w, 0] + T[w, 1])
                for j in range(n_t):
                    ci, slot = tile2call[t]
                    g = gath_tiles[ci]
                    oh = ohp.tile([128, WIN], FP16, tag="oh")
                    nc.vector.tensor_scalar(
                        oh[:], iota_sb[:],
                        dloc_sb[:, t:t + 1], val_sb[:, t:t + 1],
                        mybir.AluOpType.is_equal, mybir.AluOpType.mult,
                    )
                    nc.tensor.matmul(
                        pw[:], g[:, slot, :D], oh[:],
                        start=(j == 0), stop=(j == n_t - 1),
                    )
                    t += 1
                # sideT window -> sumXt / biXt (feature-major)
                c0 = w * WIN
                nc.vector.tensor_tensor(
                    sumxt[:D, c0:c0 + nd], egot_sb[:D, c0:c0 + nd], pw[:, :nd],
                    mybir.AluOpType.add,
                )
                nc.vector.tensor_tensor(
                    bixt[:D, c0:c0 + nd], egot_sb[:D, c0:c0 + nd], pw[:, :nd],
                    mybir.AluOpType.mult,
                )
                # padded dest columns (last window): zero side, ego=0 -> fine

            # zero the padded tail columns of sumxt/bixt (rows 0..D already
            # written only up to PER; memset the rest so matmuls see zeros)
            if PERPAD > PER:
                nc.vector.memset(sumxt[:D, PER:], 0.0)
                nc.vector.memset(bixt[:D, PER:], 0.0)

            # ---- dense tail per 128-node chunk
            for k in range(NCHUNK):
                c0 = k * 128
                p1 = pdp.tile([128, D], F32, tag="pd")
                nc.tensor.matmul(p1[:], sumxt[:, c0:c0 + 128], w1t_sb[:],
                                 start=True, stop=True)
                p2 = pdp.tile([128, D], F32, tag="pd")
                nc.tensor.matmul(p2[:], bixt[:, c0:c0 + 128], w2t_sb[:],
                                 start=True, stop=True)
                s1 = actp.tile([128, D], F32, tag="s1")
                nc.vector.tensor_scalar_mul(s1[:], p1[:], NEG_SLOPE)
                a1 = actp.tile([128, D], F32, tag="a1")
                nc.vector.tensor_tensor(a1[:], s1[:], p1[:],
                                        mybir.AluOpType.max)
                s2 = actp.tile([128, D], F32, tag="s2")
                nc.vector.tensor_scalar_mul(s2[:], p2[:], NEG_SLOPE)
                a2 = actp.tile([128, D], F32, tag="a2")
                nc.vector.tensor_tensor(a2[:], s2[:], p2[:],
                                        mybir.AluOpType.max)
                nc.vector.tensor_tensor(out_sb[:, k, :], a1[:], a2[:],
                                        mybir.AluOpType.add)

            nc.sync.dma_start(
                out.rearrange("(k p) f -> p k f", p=128), out_sb[:])

    nc.compile()
    return nc


# ---------------------------------------------------------------- entry
def kernel(ego, a_vals, W1, b1, W2, b2, a_rows, a_cols):
    ego = np.asarray(ego, dtype=np.float32)
    a_vals = np.asarray(a_vals, dtype=np.float32)
    W1 = np.asarray(W1, dtype=np.float32)
    b1 = np.asarray(b1, dtype=np.float32)
    W2 = np.asarray(W2, dtype=np.float32)
    b2 = np.asarray(b2, dtype=np.float32)
    a_rows_i = np.asarray(a_rows)
    a_cols_i = np.asarray(a_cols)

    T, per_core = _preprocess(a_rows_i, a_cols_i, a_vals)
    calls, tiles = _build_call_plan(T)

    key = (tuple(T.ravel().tolist()),)
    if key not in _CACHE:
        _CACHE[key] = _build_program(T, calls, tiles)
    nc = _CACHE[key]

    # shared inputs
    ego_pad = np.zeros((N_NODES, DPAD), dtype=np.float16)
    ego_pad[:, :D] = ego.astype(np.float16)
    iota_np = np.tile(np.arange(WIN, dtype=np.float32).astype(np.float16),
                      (128, 1))
    w1t_np = np.vstack([W1.T, b1[None, :]]).astype(np.float32)
    w2t_np = np.vstack([W2.T, b2[None, :]]).astype(np.float32)

    in_maps = []
    for c in range(NCORES):
        pc = per_core[c]
        TT = len(tiles)
        idx16_np = _wrap_idx16(pc["idx"], calls)
        val_np = pc["val"].reshape(TT, GT).T.astype(np.float32)
        dloc_np = np.ascontiguousarray(pc["dloc"].astype(np.float32).reshape(TT, GT).T)
        egot_np = np.zeros((D + 1, PERPAD), dtype=np.float32)
        egot_np[:D, :PER] = ego[c * PER:(c + 1) * PER].T
        egot_np[D, :] = 1.0
        in_maps.append({
            "ego_pad": ego_pad, "idx16": idx16_np,
            "vals": val_np, "dlocs": dloc_np, "iota": iota_np,
            "egot": egot_np, "w1t": w1t_np, "w2t": w2t_np,
        })

    res = bass_utils.run_bass_kernel_spmd(
        nc, in_maps, core_ids=list(range(NCORES)))
    global _LAST_RESULT
    _LAST_RESULT = res

    out = np.empty((N_NODES, D), dtype=np.float32)
    for c in range(NCORES):
        out[c * PER:(c + 1) * PER] = res.results[c]["out"][:PER]
    return out



# revision 2
# speedup vs baseline: 1.4165x; 1.4165x over previous
"""Trainium2 Bass kernel for KGETCDA GNN message-passing layer.

Computes, for fixed-structure inputs:
    side    = segment_sum(a_vals[:,None] * ego[a_cols], a_rows, N)
    sum_emb = LeakyReLU((ego + side) @ W1.T + b1)
    bi_emb  = LeakyReLU((ego * side) @ W2.T + b2)
    out     = sum_emb + bi_emb

Strategy (8 NeuronCores, SPMD, full inputs in / full output out):
  - Shard destination rows contiguously: core c owns rows
    [c*N/8, (c+1)*N/8).  Edges partitioned by destination.
  - Per core, edges are sorted by destination and grouped into 128-dest
    "chunks" (49 per core); each 128-edge tile gathers its source rows
    via gpsimd.dma_gather (per-edge descriptor DMA from a fp16 [N,128]
    padded copy of ego) and scatter-reduces into a 128-column slice of a
    [96, 512] PSUM window with a single matmul against a narrow [128,128]
    valued one-hot built on DVE ((iota==col)*val), fp16 with f32 PSUM.
    The narrow one-hot (vs a 512-wide one) is what keeps DVE off the
    critical path.
  - dma_gather indices are int16, so edges are split into stream A
    (src < 32768) and stream B (src >= 32768, rebased) per chunk.
  - Per-(chunk, stream) tile counts are padded to the max over the 8
    cores so the single SPMD instruction stream is valid for every core.
  - Dense tail is computed feature-major: sumXt = egoT+sideT,
    biXt = egoT*sideT (DVE, reading PSUM windows directly), then per-128-node
    chunk matmuls against bias-augmented W1T/W2T (f32), LeakyReLU on ScalarE,
    branch add on DVE, one big DMA out.
"""

import numpy as np
import ml_dtypes

import concourse.bacc as bacc
import concourse.bass as bass
import concourse.mybir as mybir
import concourse.tile as tile
from concourse import bass_utils, library_config

# ---------------------------------------------------------------- constants
N_NODES = 50000
N_EDGES = 800000
D = 96
DPAD = 128          # fp16 gather element (256B, dma_gather alignment)
NCORES = 8
PER = N_NODES // NCORES          # 6250 dests per core
CHUNK = 128                      # dests per one-hot / matmul slice
NCHUNK = (PER + CHUNK - 1) // CHUNK   # 49 chunks (last short: 106)
WIN = 512                        # dests per PSUM window (4 chunks)
NWIN = (PER + WIN - 1) // WIN    # 13 windows
SPLIT = 32768                    # int16 index limit for dma_gather
GT = 128                         # edges per tile
CT = 8                           # tiles per dma_gather call (ring limit ~1024 idxs)
NQ = 4                           # SWDGE queues
PERPAD = NCHUNK * 128            # 6272
NEG_SLOPE = 0.01

FP16 = mybir.dt.float16
F32 = mybir.dt.float32
I16 = mybir.dt.int16


# ---------------------------------------------------------------- host prep
def _preprocess(a_rows, a_cols, a_vals):
    """Per-core edge layout with core-uniform tile counts.

    Edges are bucketed by (core, chunk-of-128-dests, int16 stream) and
    padded so every core shares the per-(chunk, stream) tile count
    T[k, s] (max over cores).  Returns (T, per_core) where per_core[c]
    holds the tiled idx/val/col arrays.
    """
    a_rows = np.asarray(a_rows).astype(np.int64)
    a_cols = np.asarray(a_cols).astype(np.int64)
    a_vals = np.asarray(a_vals).astype(np.float32)

    order = np.argsort(a_rows, kind="stable")
    r_s, c_s, v_s = a_rows[order], a_cols[order], a_vals[order]
    core_s = r_s // PER
    dloc_s = r_s % PER
    chunk_s = dloc_s // CHUNK
    col_s = dloc_s % CHUNK
    stream_s = (c_s >= SPLIT).astype(np.int64)

    counts = np.zeros((NCORES, NCHUNK, 2), dtype=np.int64)
    buckets = [[[None, None] for _ in range(NCHUNK)] for _ in range(NCORES)]
    for c in range(NCORES):
        m = core_s == c
        ch, st = chunk_s[m], stream_s[m]
        src, val, col = c_s[m], v_s[m], col_s[m]
        for k in range(NCHUNK):
            mk = ch == k
            for s in range(2):
                mm = mk & (st == s)
                idx = src[mm] - (SPLIT if s else 0)
                buckets[c][k][s] = (idx, val[mm], col[mm])
                counts[c, k, s] = mm.sum()

    T = np.zeros((NCHUNK, 2), dtype=np.int64)
    for k in range(NCHUNK):
        for s in range(2):
            T[k, s] = max(1, int(np.ceil(counts[:, k, s].max() / GT)))

    per_core = []
    for c in range(NCORES):
        idx_parts, val_parts, col_parts = [], [], []
        for k in range(NCHUNK):
            for s in range(2):
                idx, val, col = buckets[c][k][s]
                n_pad = int(T[k, s]) * GT
                pad = n_pad - len(idx)
                idx_parts.append(np.concatenate([idx, np.zeros(pad, np.int64)]))
                val_parts.append(np.concatenate([val, np.zeros(pad, np.float32)]))
                col_parts.append(np.concatenate([col, np.zeros(pad, np.int64)]))
        per_core.append(dict(
            idx=np.concatenate(idx_parts),
            val=np.concatenate(val_parts).astype(np.float32),
            col=np.concatenate(col_parts),
        ))
    return T, per_core


def _build_call_plan(T):
    """Split the uniform tile sequence into dma_gather calls (<=CT tiles,
    single stream each).  Returns list of (stream, tile_start, n_tiles) in
    global tile order, plus per-tile (chunk, stream) labels."""
    calls = []
    tiles = []
    t = 0
    for k in range(NCHUNK):
        for s in range(2):
            n = int(T[k, s])
            done = 0
            while done < n:
                kk = min(CT, n - done)
                calls.append((s, t + done, kk))
                done += kk
            for _ in range(n):
                tiles.append((k, s))
            t += n
    return calls, tiles


def _wrap_idx16(idx_all, calls):
    """Per-call 16-partition-wrapped int16 index tiles, concatenated.
    Call k with n_tiles tiles occupies columns [8*tile_start, 8*(start+n))
    of a [128, 8*TT] int16 array (8 cols per tile: 128/16)."""
    TT = len(idx_all) // GT
    out = np.zeros((128, 8 * TT), dtype=np.int16)
    for s, t0, nt in calls:
        chunk = idx_all[t0 * GT:(t0 + nt) * GT].astype(np.int16)
        wrapped = chunk.reshape(-1, 16).T          # [16, nt*8]
        out[:, t0 * 8:(t0 + nt) * 8] = np.tile(wrapped, (8, 1))
    return out


# ---------------------------------------------------------------- builder
_CACHE = {}
_LAST_RESULT = None


def _build_program(T, calls, tiles):
    TT = len(tiles)
    nc = bacc.Bacc("TRN2", target_bir_lowering=False, debug=False,
                   num_devices=NCORES, num_swdge_queues=NQ)

    ego_pad = nc.dram_tensor("ego_pad", [N_NODES, DPAD], FP16, kind="ExternalInput")
    idx16 = nc.dram_tensor("idx16", [128, 8 * TT], I16, kind="ExternalInput")
    vals = nc.dram_tensor("vals", [128, TT], F32, kind="ExternalInput")
    cols = nc.dram_tensor("cols", [128, TT], F32, kind="ExternalInput")
    iota = nc.dram_tensor("iota", [128, CHUNK], FP16, kind="ExternalInput")
    egot = nc.dram_tensor("egot", [D + 1, PERPAD], F32, kind="ExternalInput")
    w1t = nc.dram_tensor("w1t", [D + 1, D], F32, kind="ExternalInput")
    w2t = nc.dram_tensor("w2t", [D + 1, D], F32, kind="ExternalInput")
    out = nc.dram_tensor("out", [PERPAD, D], F32, kind="ExternalOutput")

    tile2call = {}
    for ci, (s, t0, nt) in enumerate(calls):
        for j in range(nt):
            tile2call[t0 + j] = (ci, j)

    with tile.TileContext(nc) as tc:
        with tc.tile_pool(name="const", bufs=1) as constp, \
             tc.tile_pool(name="gath", bufs=12) as gathp, \
             tc.tile_pool(name="oh", bufs=6) as ohp, \
             tc.tile_pool(name="pw", bufs=3, space="PSUM") as pwp, \
             tc.tile_pool(name="pd", bufs=4, space="PSUM") as pdp, \
             tc.tile_pool(name="act", bufs=4) as actp, \
             tc.tile_pool(name="big", bufs=1) as bigp:

            # ---- constants / streams resident in SBUF
            idx_sb = constp.tile([128, 8 * TT], I16)
            nc.sync.dma_start(idx_sb[:], idx16[:])
            val_sb = constp.tile([128, TT], F32)
            nc.sync.dma_start(val_sb[:], vals[:])
            col_sb = constp.tile([128, TT], F32)
            nc.sync.dma_start(col_sb[:], cols[:])
            iota_sb = constp.tile([128, CHUNK], FP16)
            nc.sync.dma_start(iota_sb[:], iota[:])
            egot_sb = bigp.tile([D + 1, PERPAD], F32)
            nc.sync.dma_start(egot_sb[:], egot[:])
            w1t_sb = constp.tile([D + 1, D], F32)
            nc.sync.dma_start(w1t_sb[:], w1t[:])
            w2t_sb = constp.tile([D + 1, D], F32)
            nc.sync.dma_start(w2t_sb[:], w2t[:])

            sumxt = bigp.tile([D + 1, PERPAD], F32)
            bixt = bigp.tile([D + 1, PERPAD], F32)
            out_sb = bigp.tile([128, NCHUNK, D], F32)

            # ones rows for the bias augmentation
            nc.vector.memset(sumxt[D:D + 1, :], 1.0)
            nc.vector.memset(bixt[D:D + 1, :], 1.0)

            nc.gpsimd.load_library(library_config.mlp)

            # ---- gather calls (issued in order; Tile double-buffers)
            gath_tiles = [None] * len(calls)
            for ci, (s, t0, nt) in enumerate(calls):
                g = gathp.tile([128, CT, DPAD], FP16, tag="gath")
                src_ap = ego_pad[:SPLIT, :] if s == 0 else ego_pad[SPLIT:, :]
                nc.gpsimd.dma_gather(
                    g[:, :nt, :], src_ap, idx_sb[:, t0 * 8:(t0 + nt) * 8],
                    nt * GT, nt * GT, DPAD, queue_num=ci % NQ,
                )
                gath_tiles[ci] = g

            # ---- per-chunk accumulation + fused dense prologue per window
            t = 0
            for w in range(NWIN):
                nd = min(WIN, PER - w * WIN)
                pw = pwp.tile([D, WIN], F32, tag="pw")
                for k_sub in range(4):
                    k = 4 * w + k_sub
                    if k >= NCHUNK:
                        break
                    n_t = int(T[k, 0] + T[k, 1])
                    c0 = k_sub * CHUNK
                    for j in range(n_t):
                        ci, slot = tile2call[t]
                        g = gath_tiles[ci]
                        oh = ohp.tile([128, CHUNK], FP16, tag="oh")
                        nc.vector.tensor_scalar(
                            oh[:], iota_sb[:],
                            col_sb[:, t:t + 1], val_sb[:, t:t + 1],
                            mybir.AluOpType.is_equal, mybir.AluOpType.mult,
                        )
                        nc.tensor.matmul(
                            pw[:, c0:c0 + CHUNK], g[:, slot, :D], oh[:],
                            start=(j == 0), stop=(j == n_t - 1),
                        )
                        t += 1
                # sideT window -> sumXt / biXt (feature-major)
                c0 = w * WIN
                nc.vector.tensor_tensor(
                    sumxt[:D, c0:c0 + nd], egot_sb[:D, c0:c0 + nd], pw[:, :nd],
                    mybir.AluOpType.add,
                )
                nc.vector.tensor_tensor(
                    bixt[:D, c0:c0 + nd], egot_sb[:D, c0:c0 + nd], pw[:, :nd],
                    mybir.AluOpType.mult,
                )

            # zero the padded tail columns of sumxt/bixt
            if PERPAD > PER:
                nc.vector.memset(sumxt[:D, PER:], 0.0)
                nc.vector.memset(bixt[:D, PER:], 0.0)

            # ---- dense tail per 128-node chunk
            for k in range(NCHUNK):
                c0 = k * 128
                p1 = pdp.tile([128, D], F32, tag="pd")
                nc.tensor.matmul(p1[:], sumxt[:, c0:c0 + 128], w1t_sb[:],
                                 start=True, stop=True)
                p2 = pdp.tile([128, D], F32, tag="pd")
                nc.tensor.matmul(p2[:], bixt[:, c0:c0 + 128], w2t_sb[:],
                                 start=True, stop=True)
                s1 = actp.tile([128, D], F32, tag="s1")
                nc.vector.tensor_scalar_mul(s1[:], p1[:], NEG_SLOPE)
                a1 = actp.tile([128, D], F32, tag="a1")
                nc.vector.tensor_tensor(a1[:], s1[:], p1[:],
                                        mybir.AluOpType.max)
                s2 = actp.tile([128, D], F32, tag="s2")
                nc.vector.tensor_scalar_mul(s2[:], p2[:], NEG_SLOPE)
                a2 = actp.tile([128, D], F32, tag="a2")
                nc.vector.tensor_tensor(a2[:], s2[:], p2[:],
                                        mybir.AluOpType.max)
                nc.vector.tensor_tensor(out_sb[:, k, :], a1[:], a2[:],
                                        mybir.AluOpType.add)

            nc.sync.dma_start(
                out.rearrange("(k p) f -> p k f", p=128), out_sb[:])

    nc.compile()
    return nc


# ---------------------------------------------------------------- entry
def kernel(ego, a_vals, W1, b1, W2, b2, a_rows, a_cols):
    ego = np.asarray(ego, dtype=np.float32)
    a_vals = np.asarray(a_vals, dtype=np.float32)
    W1 = np.asarray(W1, dtype=np.float32)
    b1 = np.asarray(b1, dtype=np.float32)
    W2 = np.asarray(W2, dtype=np.float32)
    b2 = np.asarray(b2, dtype=np.float32)
    a_rows_i = np.asarray(a_rows)
    a_cols_i = np.asarray(a_cols)

    T, per_core = _preprocess(a_rows_i, a_cols_i, a_vals)
    calls, tiles = _build_call_plan(T)

    key = (tuple(T.ravel().tolist()),)
    if key not in _CACHE:
        _CACHE[key] = _build_program(T, calls, tiles)
    nc = _CACHE[key]

    # shared inputs
    ego_pad = np.zeros((N_NODES, DPAD), dtype=np.float16)
    ego_pad[:, :D] = ego.astype(np.float16)
    iota_np = np.tile(np.arange(CHUNK, dtype=np.float32).astype(np.float16),
                      (128, 1))
    w1t_np = np.vstack([W1.T, b1[None, :]]).astype(np.float32)
    w2t_np = np.vstack([W2.T, b2[None, :]]).astype(np.float32)

    in_maps = []
    TT = len(tiles)
    for c in range(NCORES):
        pc = per_core[c]
        idx16_np = _wrap_idx16(pc["idx"], calls)
        val_np = pc["val"].reshape(TT, GT).T.astype(np.float32)
        col_np = np.ascontiguousarray(
            pc["col"].astype(np.float32).reshape(TT, GT).T)
        egot_np = np.zeros((D + 1, PERPAD), dtype=np.float32)
        egot_np[:D, :PER] = ego[c * PER:(c + 1) * PER].T
        egot_np[D, :] = 1.0
        in_maps.append({
            "ego_pad": ego_pad, "idx16": idx16_np,
            "vals": val_np, "cols": col_np, "iota": iota_np,
            "egot": egot_np, "w1t": w1t_np, "w2t": w2t_np,
        })

    res = bass_utils.run_bass_kernel_spmd(
        nc, in_maps, core_ids=list(range(NCORES)))
    global _LAST_RESULT
    _LAST_RESULT = res

    out = np.empty((N_NODES, D), dtype=np.float32)
    for c in range(NCORES):
        out[c * PER:(c + 1) * PER] = res.results[c]["out"][:PER]
    return out
